# revision 17
# baseline (speedup 1.0000x reference)
"""Multi-head attention kernel for Trainium2 (Bass/Tile), 8-core SPMD.

Problem: Q,K,V [B=2, H=16, S=4096, D=64] fp32 -> softmax(Q K^T / sqrt(D)) V.
Sharding: batch*heads (32) split 4-per-core across 8 NeuronCores; each core
computes its heads independently (no collectives).

Per-head algorithm (transposed-scores flash attention, fp16 matmuls, P
production split across two engines):
  scoresT[k,q] = K[k,:] . Q[q,:]        (PE, fp16 operands, fp32 PSUM,
                                         row-tiled pairs: two k-chunks run
                                         concurrently in the 128x128 array)
  pT[k,q]     = exp(scoresT / 8)        (fp16; per 3-chunk group either ACT
                                         exact exp, or DVE fp16-Schraudolph:
                                         u16 = round(s*1024*log2(e)/8 + b)
                                         bitcast as fp16 == 2^x with a
                                         mean-compensated ~+-3% mantissa-
                                         interpolation sawtooth that washes
                                         out in the softmax normalization)
  accT[d,q]  += Vaug[k,d] . pT[k,q]     (PE fp16; Vaug row 64 == ones, so acc
                                         row 64 accumulates the denominator)
  oT[d,q]     = accT[d,q] * (1/accT[64,q])
                                        (tracked DVE copy of the Z row ->
                                         reciprocal_approx_fast (custom DVE
                                         ops must not read live PSUM) ->
                                         GpSimd partition broadcast -> DVE
                                         multiply, fp16 out)
Host side only re-lays-out data: QT/KT transposed per head, V augmented with
a ones column, output OT transposed back and gathered.
"""

import numpy as np
from contextlib import ExitStack

import concourse.bacc as bacc
import concourse.bass as bass
import concourse.tile as tile
import concourse.mybir as mybir
from concourse.bass_utils import run_bass_kernel_spmd

F32 = mybir.dt.float32
F16 = mybir.dt.float16
U16 = mybir.dt.uint16
EXP = mybir.ActivationFunctionType.Exp
MULT = mybir.AluOpType.mult
ADD = mybir.AluOpType.add

B, H, S, D = 2, 16, 4096, 64
N_CORES = 8
HPC = (B * H) // N_CORES  # heads per core

QTILE = 512            # q columns processed per inner iteration
CHUNK = 128            # k rows per matmul (PE partition dim)
GROUP = 2              # k-chunks exp'd per ACT/DVE instruction

# fp16 Schraudolph: u16 = round(score * 1024*log2(e)/8 + bias); the bias is
# 15*1024 minus 58.7 to zero the mean of the 2^f vs (1+f) mantissa sawtooth
DVE_MUL = 184.66496
DVE_ADD = 15301.3
# groups (of 16 per qtile) whose P runs on DVE instead of ACT (~14/32 chunks);
# odd groups -> each 2-group block pairs one ACT with one DVE producer
DVE_GROUPS = frozenset({1, 3, 5, 7, 9, 11, 13})
FLUSH_DEPTH = 2        # MM2 groups held back so PE never queues behind P


def build_nc(hpc: int = HPC, s: int = S, qtile: int = QTILE):
    n_chunks = s // CHUNK
    n_qtiles = s // qtile
    group_sizes = [GROUP] * (n_chunks // GROUP)
    if n_chunks % GROUP:
        group_sizes.append(n_chunks % GROUP)

    nc = bacc.Bacc("TRN2", target_bir_lowering=False, debug=False)
    qt_d = nc.dram_tensor("qt", [hpc, D, s], F16, kind="ExternalInput").ap()
    kt_d = nc.dram_tensor("kt", [hpc, D, s], F16, kind="ExternalInput").ap()
    va_d = nc.dram_tensor("va", [hpc, s, D + 1], F16, kind="ExternalInput").ap()
    o_d = nc.dram_tensor("o", [hpc, D, s], F16, kind="ExternalOutput").ap()

    with tile.TileContext(nc) as tc, ExitStack() as ctx:
        qk_pool = ctx.enter_context(tc.tile_pool(name="qk", bufs=2))
        v_pool = ctx.enter_context(tc.tile_pool(name="v", bufs=2))
        pt_pool = ctx.enter_context(tc.tile_pool(name="pt", bufs=6))
        ot_pool = ctx.enter_context(tc.tile_pool(name="ot", bufs=2))
        small_pool = ctx.enter_context(tc.tile_pool(name="small", bufs=4))
        const_pool = ctx.enter_context(tc.tile_pool(name="const", bufs=1))
        sc_psum = ctx.enter_context(tc.tile_pool(name="sc", bufs=3, space="PSUM"))
        oa_psum = ctx.enter_context(tc.tile_pool(name="oa", bufs=2, space="PSUM"))

        # prewarm the ACT exp table set while the first DMAs are in flight
        warm = const_pool.tile([1, 1], F32)
        nc.vector.memset(warm[:], 0.0)
        warm2 = const_pool.tile([1, 1], F32)
        nc.scalar.activation(warm2[:], warm[:], EXP, scale=1.0)
        # PE warm-up burst: ~4us of tiny matmuls during the initial DMA wait
        # flips the HAM clock gate to 8/8 (2.4 GHz) before real work arrives
        wsrc = const_pool.tile([64, 128], F16)
        nc.vector.memset(wsrc.bitcast(U16)[:], 0)
        wp = sc_psum.tile([128, GROUP, QTILE], F32, tag="sc")
        for _ in range(170):
            nc.tensor.matmul(wp[0:128, 0, 0:16], wsrc[:], wsrc[:, 0:16],
                             start=True, stop=True)

        # software-pipelined emission: each group's PV matmuls (MM2) are
        # deferred until after the NEXT group's score matmuls (MM1) and P
        # production -- across qtile AND head boundaries -- so the next P is
        # never queued on PE behind MM2s that wait on the current P.
        pending = []  # deque of (h, qt, acc, chunks, p_t, va_sb, o_t, is_last)
        epi_pending = []  # deferred stage-B epilogues: (h, qt, acc, bc, o_t)

        def epilogue_a(h_, qt_, acc_, o_t_):
            # stage A: pull Z out of PSUM with a *tracked* DVE copy (waits
            # for the accumulation stop); the custom-op fast reciprocal then
            # reads SBUF only and is FIFO-ordered behind the copy -- custom
            # DVE ops must never read live PSUM (their reads are not
            # dependency-tracked against PE accumulation).
            zrow = small_pool.tile([1, qtile], F32, tag="zrow")
            nc.vector.tensor_copy(zrow[:], acc_[D : D + 1, :])
            dinv = small_pool.tile([1, qtile], F32, tag="dinv")
            nc.vector.reciprocal_approx_fast(dinv[:], zrow[:])
            bc = small_pool.tile([D, qtile], F32, tag="bc")
            nc.gpsimd.partition_broadcast(bc[:], dinv[:])
            epi_pending.append((h_, qt_, acc_, bc, o_t_))

        def epilogue_b():
            # stage B (deferred a few groups so the DVE FIFO never waits on
            # the GpSimd broadcast): o = numerator * (1/Z), fp16 out
            h_, qt_, acc_, bc_, o_t_ = epi_pending.pop(0)
            qs_ = slice(qt_ * qtile, (qt_ + 1) * qtile)
            nc.vector.tensor_mul(o_t_[:, qs_], acc_[0:D, :], bc_[:])
            # stream the finished qtile out on the SWDGE path so input
            # loads on the HWDGE ring are never queued behind stores
            nc.gpsimd.dma_start(o_d[h_][:, qs_], o_t_[:, qs_])

        def flush_one():
            h_, qt_, acc_, chunks_, pt_, va_, ot_, last_ = pending.pop(0)
            for j, c in enumerate(chunks_):
                nc.tensor.matmul(
                    acc_[:], va_[:, c, :], pt_[:, j, :],
                    start=(c == 0), stop=(c == n_chunks - 1),
                )
            if last_:
                epilogue_a(h_, qt_, acc_, ot_)

        def flush_pending(depth=0):
            while len(pending) > depth:
                flush_one()

        for h in range(hpc):
            # K^T and Q^T [D, s] duplicated into both partition halves so two
            # k-chunks can run concurrently via PE row tiling.
            qt_sb = qk_pool.tile([128, s], F16, tag="qt")
            kt_sb = qk_pool.tile([128, s], F16, tag="kt")
            va_sb = v_pool.tile([128, n_chunks, D + 1], F16)
            va_r = va_d[h].rearrange("(c p) e -> p c e", p=128)
            # tiered loads: small leading slices of everything first, then
            # interleaved k/V column pieces (k columns are consumed in order
            # by the chunk loop), with the q tails (needed only from qtile 1)
            # last
            kcut = min(8 * CHUNK, s)
            ncut = kcut // CHUNK
            nc.sync.dma_start(kt_sb[0:D, 0:kcut], kt_d[h][:, 0:kcut])
            nc.sync.dma_start(qt_sb[0:D, 0:qtile], qt_d[h][:, 0:qtile])
            nc.sync.dma_start(kt_sb[D : 2 * D, 0:kcut], kt_d[h][:, 0:kcut])
            nc.sync.dma_start(qt_sb[D : 2 * D, 0:qtile], qt_d[h][:, 0:qtile])
            nc.sync.dma_start(va_sb[:, 0:ncut, :], va_r[:, 0:ncut, :])
            cuts = [kcut]
            while cuts[-1] < s:
                cuts.append(min(cuts[-1] + 12 * CHUNK, s))
            for c0_, c1_ in zip(cuts, cuts[1:]):
                n0_, n1_ = c0_ // CHUNK, c1_ // CHUNK
                nc.sync.dma_start(kt_sb[0:D, c0_:c1_], kt_d[h][:, c0_:c1_])
                nc.sync.dma_start(kt_sb[D : 2 * D, c0_:c1_], kt_d[h][:, c0_:c1_])
                nc.sync.dma_start(va_sb[:, n0_:n1_, :], va_r[:, n0_:n1_, :])
            if qtile < s:
                nc.sync.dma_start(qt_sb[0:D, qtile:s], qt_d[h][:, qtile:s])
                nc.sync.dma_start(qt_sb[D : 2 * D, qtile:s], qt_d[h][:, qtile:s])
            o_t = ot_pool.tile([D, s], F16)

            for qt in range(n_qtiles):
                qs = slice(qt * qtile, (qt + 1) * qtile)
                acc = oa_psum.tile([D + 1, qtile], F32)
                n_groups = len(group_sizes)
                # emit in blocks of two groups: MM1 x4, then both P ops (one
                # ACT + one DVE, concurrent), then the previous block's MM2
                # x4 -- longer same-weight-source runs cost fewer PE weight
                # switches than strict per-group alternation
                for m in range(n_groups // 2):
                    blk = []
                    for gb in range(2):
                        gi = 2 * m + gb
                        gs = group_sizes[gi]
                        chunks = list(range(GROUP * gi, GROUP * gi + gs))
                        sc = sc_psum.tile([128, GROUP, qtile], F32, tag="sc")
                        for j, c in enumerate(chunks):
                            # alternate partition halves by global chunk index
                            # so consecutive chunks always run concurrently
                            # via PE row tiling, across group boundaries too
                            half = slice(0, D) if c % 2 == 0 else slice(D, 2 * D)
                            nc.tensor.matmul(
                                sc[:, j, :],
                                kt_sb[half, c * CHUNK : (c + 1) * CHUNK],
                                qt_sb[half, qs],
                                start=True, stop=True,
                            )
                        blk.append((gi, gs, chunks, sc))
                    for gi, gs, chunks, sc in blk:
                        p_t = pt_pool.tile([128, GROUP, qtile], F16, tag="pt")
                        if gi in DVE_GROUPS:
                            nc.vector.tensor_scalar(
                                p_t.bitcast(U16)[:, 0:gs, :], sc[:, 0:gs, :],
                                DVE_MUL, DVE_ADD, MULT, ADD,
                            )
                        else:
                            nc.scalar.activation(
                                p_t[:, 0:gs, :], sc[:, 0:gs, :], EXP, scale=0.125
                            )
                        pending.append((
                            h, qt, acc, chunks, p_t, va_sb, o_t,
                            gi == n_groups - 1,
                        ))
                    flush_pending(FLUSH_DEPTH)
                    if epi_pending and m == 2:
                        epilogue_b()
        flush_pending()
        while epi_pending:
            epilogue_b()

    nc.compile()
    return nc


_NC_CACHE = {}


def _get_nc(hpc=HPC, s=S, qtile=QTILE):
    key = (hpc, s, qtile)
    if key not in _NC_CACHE:
        _NC_CACHE[key] = build_nc(hpc, s, qtile)
    return _NC_CACHE[key]


def prep_inputs(Q, K, V):
    """Host-side re-layout: per-core input maps."""
    bh = B * H
    q2 = np.ascontiguousarray(
        np.asarray(Q, dtype=np.float32).reshape(bh, S, D).transpose(0, 2, 1)
    ).astype(np.float16)
    k2 = np.ascontiguousarray(
        np.asarray(K, dtype=np.float32).reshape(bh, S, D).transpose(0, 2, 1)
    ).astype(np.float16)
    v = np.asarray(V, dtype=np.float32).reshape(bh, S, D).astype(np.float16)
    va = np.concatenate([v, np.ones((bh, S, 1), dtype=np.float16)], axis=-1)
    in_maps = []
    for c in range(N_CORES):
        sl = slice(c * HPC, (c + 1) * HPC)
        in_maps.append({
            "qt": np.ascontiguousarray(q2[sl]),
            "kt": np.ascontiguousarray(k2[sl]),
            "va": np.ascontiguousarray(va[sl]),
        })
    return in_maps


def run(Q, K, V, trace=False, **kwargs):
    nc = _get_nc()
    in_maps = prep_inputs(Q, K, V)
    res = run_bass_kernel_spmd(
        nc, in_maps, core_ids=list(range(N_CORES)), trace=trace, **kwargs
    )
    # o is [hpc, D, s] fp16 per core -> transpose back to [hpc, s, D] fp32
    outs = [
        np.ascontiguousarray(
            res.results[c]["o"].transpose(0, 2, 1).astype(np.float32)
        )
        for c in range(N_CORES)
    ]
    full = np.concatenate(outs, axis=0).reshape(B, H, S, D)
    return full, res


def kernel(Q, K, V):
    # retry on transient device/runtime errors (e.g. a wedged NeuronCore
    # left over from a previous run recovers on re-execution)
    import time
    last = None
    for attempt in range(3):
        try:
            out, _ = run(Q, K, V)
            return out
        except Exception as e:  # noqa: BLE001
            last = e
            time.sleep(5)
    raise last


# revision 18
# speedup vs baseline: 1.0056x; 1.0056x over previous
"""Multi-head attention kernel for Trainium2 (Bass/Tile), 8-core SPMD.

Problem: Q,K,V [B=2, H=16, S=4096, D=64] fp32 -> softmax(Q K^T / sqrt(D)) V.
Sharding: batch*heads (32) split 4-per-core across 8 NeuronCores; each core
computes its heads independently (no collectives).

Per-head algorithm (transposed-scores flash attention, fp16 matmuls, P
production split across two engines):
  scoresT[k,q] = K[k,:] . Q[q,:]        (PE, fp16 operands, fp32 PSUM,
                                         row-tiled pairs: two k-chunks run
                                         concurrently in the 128x128 array)
  pT[k,q]     = exp(scoresT / 8)        (fp16; per 3-chunk group either ACT
                                         exact exp, or DVE fp16-Schraudolph:
                                         u16 = round(s*1024*log2(e)/8 + b)
                                         bitcast as fp16 == 2^x with a
                                         mean-compensated ~+-3% mantissa-
                                         interpolation sawtooth that washes
                                         out in the softmax normalization)
  accT[d,q]  += Vaug[k,d] . pT[k,q]     (PE fp16; Vaug row 64 == ones, so acc
                                         row 64 accumulates the denominator)
  oT[d,q]     = accT[d,q] * (1/accT[64,q])
                                        (tracked DVE copy of the Z row ->
                                         reciprocal_approx_fast (custom DVE
                                         ops must not read live PSUM) ->
                                         GpSimd partition broadcast -> DVE
                                         multiply, fp16 out)
Host side only re-lays-out data: QT/KT transposed per head, V augmented with
a ones column, output OT transposed back and gathered.
"""

import numpy as np
from contextlib import ExitStack

import concourse.bacc as bacc
import concourse.bass as bass
import concourse.tile as tile
import concourse.mybir as mybir
from concourse.bass_utils import run_bass_kernel_spmd

F32 = mybir.dt.float32
F16 = mybir.dt.float16
U16 = mybir.dt.uint16
EXP = mybir.ActivationFunctionType.Exp
MULT = mybir.AluOpType.mult
ADD = mybir.AluOpType.add

B, H, S, D = 2, 16, 4096, 64
N_CORES = 8
HPC = (B * H) // N_CORES  # heads per core

QTILE = 512            # q columns processed per inner iteration
CHUNK = 128            # k rows per matmul (PE partition dim)
GROUP = 2              # k-chunks exp'd per ACT/DVE instruction

# fp16 Schraudolph: u16 = round(score * 1024*log2(e)/8 + bias); the bias is
# 15*1024 minus 58.7 to zero the mean of the 2^f vs (1+f) mantissa sawtooth
DVE_MUL = 184.66496
DVE_ADD = 15301.3
# groups (of 16 per qtile) whose P runs on DVE instead of ACT (~14/32 chunks);
# odd groups -> each 2-group block pairs one ACT with one DVE producer
DVE_GROUPS = frozenset({1, 3, 7, 9, 11, 13, 15})
FLUSH_DEPTH = 2        # MM2 groups held back so PE never queues behind P


def build_nc(hpc: int = HPC, s: int = S, qtile: int = QTILE):
    n_chunks = s // CHUNK
    n_qtiles = s // qtile
    group_sizes = [GROUP] * (n_chunks // GROUP)
    if n_chunks % GROUP:
        group_sizes.append(n_chunks % GROUP)

    nc = bacc.Bacc("TRN2", target_bir_lowering=False, debug=False)
    qt_d = nc.dram_tensor("qt", [hpc, D, s], F16, kind="ExternalInput").ap()
    kt_d = nc.dram_tensor("kt", [hpc, D, s], F16, kind="ExternalInput").ap()
    va_d = nc.dram_tensor("va", [hpc, s, D + 1], F16, kind="ExternalInput").ap()
    o_d = nc.dram_tensor("o", [hpc, D, s], F16, kind="ExternalOutput").ap()

    with tile.TileContext(nc) as tc, ExitStack() as ctx:
        qk_pool = ctx.enter_context(tc.tile_pool(name="qk", bufs=2))
        v_pool = ctx.enter_context(tc.tile_pool(name="v", bufs=2))
        pt_pool = ctx.enter_context(tc.tile_pool(name="pt", bufs=6))
        ot_pool = ctx.enter_context(tc.tile_pool(name="ot", bufs=2))
        small_pool = ctx.enter_context(tc.tile_pool(name="small", bufs=4))
        const_pool = ctx.enter_context(tc.tile_pool(name="const", bufs=1))
        sc_psum = ctx.enter_context(tc.tile_pool(name="sc", bufs=3, space="PSUM"))
        oa_psum = ctx.enter_context(tc.tile_pool(name="oa", bufs=2, space="PSUM"))

        # prewarm the ACT exp table set while the first DMAs are in flight
        warm = const_pool.tile([1, 1], F32)
        nc.vector.memset(warm[:], 0.0)
        warm2 = const_pool.tile([1, 1], F32)
        nc.scalar.activation(warm2[:], warm[:], EXP, scale=1.0)
        # PE warm-up burst: ~4us of tiny matmuls during the initial DMA wait
        # flips the HAM clock gate to 8/8 (2.4 GHz) before real work arrives
        wsrc = const_pool.tile([64, 128], F16)
        nc.vector.memset(wsrc.bitcast(U16)[:], 0)
        wp = sc_psum.tile([128, GROUP, QTILE], F32, tag="sc")
        for _ in range(170):
            nc.tensor.matmul(wp[0:128, 0, 0:16], wsrc[:], wsrc[:, 0:16],
                             start=True, stop=True)

        # software-pipelined emission: each group's PV matmuls (MM2) are
        # deferred until after the NEXT group's score matmuls (MM1) and P
        # production -- across qtile AND head boundaries -- so the next P is
        # never queued on PE behind MM2s that wait on the current P.
        pending = []  # deque of (h, qt, acc, chunks, p_t, va_sb, o_t, is_last)
        epi_pending = []  # deferred stage-B epilogues: (h, qt, acc, bc, o_t)

        def epilogue_a(h_, qt_, acc_, o_t_):
            # stage A: pull Z out of PSUM with a *tracked* DVE copy (waits
            # for the accumulation stop); the custom-op fast reciprocal then
            # reads SBUF only and is FIFO-ordered behind the copy -- custom
            # DVE ops must never read live PSUM (their reads are not
            # dependency-tracked against PE accumulation).
            zrow = small_pool.tile([1, qtile], F32, tag="zrow")
            nc.vector.tensor_copy(zrow[:], acc_[D : D + 1, :])
            dinv = small_pool.tile([1, qtile], F32, tag="dinv")
            nc.vector.reciprocal_approx_fast(dinv[:], zrow[:])
            bc = small_pool.tile([D, qtile], F32, tag="bc")
            nc.gpsimd.partition_broadcast(bc[:], dinv[:])
            epi_pending.append((h_, qt_, acc_, bc, o_t_))

        def epilogue_b():
            # stage B (deferred a few groups so the DVE FIFO never waits on
            # the GpSimd broadcast): o = numerator * (1/Z), fp16 out
            h_, qt_, acc_, bc_, o_t_ = epi_pending.pop(0)
            qs_ = slice(qt_ * qtile, (qt_ + 1) * qtile)
            nc.vector.tensor_mul(o_t_[:, qs_], acc_[0:D, :], bc_[:])
            # stream the finished qtile out on the SWDGE path so input
            # loads on the HWDGE ring are never queued behind stores
            nc.gpsimd.dma_start(o_d[h_][:, qs_], o_t_[:, qs_])

        def flush_one():
            h_, qt_, acc_, chunks_, pt_, va_, ot_, last_ = pending.pop(0)
            for j, c in enumerate(chunks_):
                nc.tensor.matmul(
                    acc_[:], va_[:, c, :], pt_[:, j, :],
                    start=(c == 0), stop=(c == n_chunks - 1),
                )
            if last_:
                epilogue_a(h_, qt_, acc_, ot_)

        def flush_pending(depth=0):
            while len(pending) > depth:
                flush_one()

        for h in range(hpc):
            # K^T and Q^T [D, s] duplicated into both partition halves so two
            # k-chunks can run concurrently via PE row tiling.
            qt_sb = qk_pool.tile([128, s], F16, tag="qt")
            kt_sb = qk_pool.tile([128, s], F16, tag="kt")
            va_sb = v_pool.tile([128, n_chunks, D + 1], F16)
            va_r = va_d[h].rearrange("(c p) e -> p c e", p=128)
            # tiered loads: small leading slices of everything first, then
            # interleaved k/V column pieces (k columns are consumed in order
            # by the chunk loop), with the q tails (needed only from qtile 1)
            # last
            kcut = min(8 * CHUNK, s)
            ncut = kcut // CHUNK
            nc.sync.dma_start(kt_sb[0:D, 0:kcut], kt_d[h][:, 0:kcut])
            nc.sync.dma_start(qt_sb[0:D, 0:qtile], qt_d[h][:, 0:qtile])
            nc.sync.dma_start(kt_sb[D : 2 * D, 0:kcut], kt_d[h][:, 0:kcut])
            nc.sync.dma_start(qt_sb[D : 2 * D, 0:qtile], qt_d[h][:, 0:qtile])
            nc.sync.dma_start(va_sb[:, 0:ncut, :], va_r[:, 0:ncut, :])
            cuts = [kcut]
            while cuts[-1] < s:
                cuts.append(min(cuts[-1] + 12 * CHUNK, s))
            for c0_, c1_ in zip(cuts, cuts[1:]):
                n0_, n1_ = c0_ // CHUNK, c1_ // CHUNK
                nc.sync.dma_start(kt_sb[0:D, c0_:c1_], kt_d[h][:, c0_:c1_])
                nc.sync.dma_start(kt_sb[D : 2 * D, c0_:c1_], kt_d[h][:, c0_:c1_])
                nc.sync.dma_start(va_sb[:, n0_:n1_, :], va_r[:, n0_:n1_, :])
            if qtile < s:
                nc.sync.dma_start(qt_sb[0:D, qtile:s], qt_d[h][:, qtile:s])
                nc.sync.dma_start(qt_sb[D : 2 * D, qtile:s], qt_d[h][:, qtile:s])
            o_t = ot_pool.tile([D, s], F16)

            for qt in range(n_qtiles):
                qs = slice(qt * qtile, (qt + 1) * qtile)
                acc = oa_psum.tile([D + 1, qtile], F32)
                n_groups = len(group_sizes)
                # emit in blocks of two groups: MM1 x4, then both P ops (one
                # ACT + one DVE, concurrent), then the previous block's MM2
                # x4 -- longer same-weight-source runs cost fewer PE weight
                # switches than strict per-group alternation
                for m in range(n_groups // 2):
                    blk = []
                    for gb in range(2):
                        gi = 2 * m + gb
                        gs = group_sizes[gi]
                        chunks = list(range(GROUP * gi, GROUP * gi + gs))
                        sc = sc_psum.tile([128, GROUP, qtile], F32, tag="sc")
                        for j, c in enumerate(chunks):
                            # alternate partition halves by global chunk index
                            # so consecutive chunks always run concurrently
                            # via PE row tiling, across group boundaries too
                            half = slice(0, D) if c % 2 == 0 else slice(D, 2 * D)
                            nc.tensor.matmul(
                                sc[:, j, :],
                                kt_sb[half, c * CHUNK : (c + 1) * CHUNK],
                                qt_sb[half, qs],
                                start=True, stop=True,
                            )
                        blk.append((gi, gs, chunks, sc))
                    for gi, gs, chunks, sc in blk:
                        p_t = pt_pool.tile([128, GROUP, qtile], F16, tag="pt")
                        if gi in DVE_GROUPS:
                            nc.vector.tensor_scalar(
                                p_t.bitcast(U16)[:, 0:gs, :], sc[:, 0:gs, :],
                                DVE_MUL, DVE_ADD, MULT, ADD,
                            )
                        else:
                            nc.scalar.activation(
                                p_t[:, 0:gs, :], sc[:, 0:gs, :], EXP, scale=0.125
                            )
                        pending.append((
                            h, qt, acc, chunks, p_t, va_sb, o_t,
                            gi == n_groups - 1,
                        ))
                    flush_pending(FLUSH_DEPTH)
                    if epi_pending and m == 2:
                        epilogue_b()
        flush_pending()
        while epi_pending:
            epilogue_b()

    nc.compile()
    return nc


_NC_CACHE = {}


def _get_nc(hpc=HPC, s=S, qtile=QTILE):
    key = (hpc, s, qtile)
    if key not in _NC_CACHE:
        _NC_CACHE[key] = build_nc(hpc, s, qtile)
    return _NC_CACHE[key]


def prep_inputs(Q, K, V):
    """Host-side re-layout: per-core input maps."""
    bh = B * H
    q2 = np.ascontiguousarray(
        np.asarray(Q, dtype=np.float32).reshape(bh, S, D).transpose(0, 2, 1)
    ).astype(np.float16)
    k2 = np.ascontiguousarray(
        np.asarray(K, dtype=np.float32).reshape(bh, S, D).transpose(0, 2, 1)
    ).astype(np.float16)
    v = np.asarray(V, dtype=np.float32).reshape(bh, S, D).astype(np.float16)
    va = np.concatenate([v, np.ones((bh, S, 1), dtype=np.float16)], axis=-1)
    in_maps = []
    for c in range(N_CORES):
        sl = slice(c * HPC, (c + 1) * HPC)
        in_maps.append({
            "qt": np.ascontiguousarray(q2[sl]),
            "kt": np.ascontiguousarray(k2[sl]),
            "va": np.ascontiguousarray(va[sl]),
        })
    return in_maps


def run(Q, K, V, trace=False, **kwargs):
    nc = _get_nc()
    in_maps = prep_inputs(Q, K, V)
    res = run_bass_kernel_spmd(
        nc, in_maps, core_ids=list(range(N_CORES)), trace=trace, **kwargs
    )
    # o is [hpc, D, s] fp16 per core -> transpose back to [hpc, s, D] fp32
    outs = [
        np.ascontiguousarray(
            res.results[c]["o"].transpose(0, 2, 1).astype(np.float32)
        )
        for c in range(N_CORES)
    ]
    full = np.concatenate(outs, axis=0).reshape(B, H, S, D)
    return full, res


def kernel(Q, K, V):
    # retry on transient device/runtime errors (e.g. a wedged NeuronCore
    # left over from a previous run recovers on re-execution)
    import time
    last = None
    for attempt in range(3):
        try:
            out, _ = run(Q, K, V)
            return out
        except Exception as e:  # noqa: BLE001
            last = e
            time.sleep(5)
    raise last
